# revision 1
# baseline (speedup 1.0000x reference)
import sys

for p in ("/opt/trn_rl_repo",):
    if p not in sys.path:
        sys.path.insert(0, p)

import numpy as np
import ml_dtypes

import concourse.bass as bass
from concourse import bacc
import concourse.mybir as mybir
import concourse.tile as tile
from concourse.bass import ds, ts
from concourse.bass_utils import run_bass_kernel_spmd

BF16 = ml_dtypes.bfloat16

B, N, DIM, NH = 256, 196, 256, 8
HD = DIM // NH  # 32
G = 14
NCORES = 8
BLOC = B // NCORES  # 32
NC2 = 98  # N / 2
SLAB = 8  # batches per input-DMA slab
NSLAB = BLOC // SLAB
RPB_R = 32  # rank of the rpb factorization folded into the QK matmul
KEXT = HD + RPB_R  # 64


def _relative_position_index(g: int) -> np.ndarray:
    coords = np.stack(np.meshgrid(np.arange(g), np.arange(g), indexing="ij"))
    cf = coords.reshape(2, -1)
    rel = cf[:, :, None] - cf[:, None, :]
    rel = rel.transpose(1, 2, 0).astype(np.int64)
    rel[..., 0] += g - 1
    rel[..., 1] += g - 1
    rel[..., 0] *= 2 * g - 1
    return rel.sum(-1)


def _bias_coords(g: int) -> np.ndarray:
    p = np.arange(1 - g, g)
    biases = np.stack(np.meshgrid(p, p, indexing="ij"))
    return biases.reshape(2, -1).T.astype(np.float32)


_CACHED = {}


def _build_bass(reps: int = 1):
    key = ("nc", reps)
    if key in _CACHED:
        return _CACHED[key]
    f32 = mybir.dt.float32
    bf16 = mybir.dt.bfloat16

    nc = bacc.Bacc("TRN2", target_bir_lowering=False)
    # partition-major packed inputs; q/k carry 32 extra contraction rows
    # holding the rank-32 factorization of the relative position bias.
    # q and k share one tensor so each chunk needs a single DMA.
    qk_d = nc.dram_tensor("qk", [2, KEXT, BLOC, 8, 196], bf16, kind="ExternalInput")
    vx_d = nc.dram_tensor("vx", [NC2, BLOC, 2, 8, 33], bf16, kind="ExternalInput")
    w_d = nc.dram_tensor("w", [128, 2, 256], bf16, kind="ExternalInput")
    pb_d = nc.dram_tensor("pb", [1, 256], bf16, kind="ExternalInput")
    id_d = nc.dram_tensor("ident", [NC2, NC2], bf16, kind="ExternalInput")
    out_d = nc.dram_tensor("out", [BLOC, 196, 256], bf16, kind="ExternalOutput")

    from contextlib import ExitStack

    with tile.TileContext(nc) as tc, ExitStack() as es:
        const = es.enter_context(tc.tile_pool(name="const", bufs=1))
        io = es.enter_context(tc.tile_pool(name="io", bufs=3))
        work = es.enter_context(tc.tile_pool(name="work", bufs=6))
        ps_pool = es.enter_context(tc.tile_pool(name="ps", bufs=2, space="PSUM"))
        px_pool = es.enter_context(tc.tile_pool(name="px", bufs=1, space="PSUM"))
        pt_pool = es.enter_context(tc.tile_pool(name="pt", bufs=1, space="PSUM"))
        po_pool = es.enter_context(tc.tile_pool(name="po", bufs=1, space="PSUM"))

        w_sb = const.tile([128, 2, 256], bf16)
        pb_sb = const.tile([1, 256], bf16)
        id_sb = const.tile([NC2, NC2], bf16)
        ones_sb = const.tile([1, NC2], bf16)
        nc.vector.memset(ones_sb[:], 1.0)

        def _issue_const_dmas():
            nc.sync.dma_start(w_sb[:], w_d[:])
            nc.sync.dma_start(pb_sb[:], pb_d[:])
            nc.sync.dma_start(id_sb[:], id_d[:])

        for rep in range(reps):
            def _issue_slab(s, first=False):
                qk_sb = io.tile([KEXT, 2, SLAB, 8, 196], bf16, tag="qk")
                vx_sb = io.tile([NC2, SLAB, 2, 8, 33], bf16, tag="vx")
                # chunked DMAs: data availability tracks per-batch compute
                # instead of stalling on whole-slab transfers
                chunks = (
                    [(0, 1), (1, 1), (2, 2), (4, 2), (6, 2)]
                    if first
                    else [(0, 2), (2, 2), (4, 2), (6, 2)]
                )
                for ci, (c0, cn) in enumerate(chunks):
                    nc.sync.dma_start(
                        qk_sb[:, :, ds(c0, cn)].rearrange("p q b h n -> p q (b h n)"),
                        qk_d[:, :, ds(s * SLAB + c0, cn)].rearrange(
                            "q p b h n -> p q (b h n)"
                        ),
                    )
                    nc.sync.dma_start(
                        vx_sb[:, ds(c0, cn)].rearrange("p b j h v -> p (b j h v)"),
                        vx_d[:, ds(s * SLAB + c0, cn)].rearrange("p b j h v -> p (b j h v)"),
                    )
                    if first and ci == 1:
                        _issue_const_dmas()
                return qk_sb, vx_sb

            pending = _issue_slab(0, first=(rep == 0))
            for s in range(NSLAB):
                qk_sb, vx_sb = pending

                for bi in range(SLAB):
                    if bi == 1 and s + 1 < NSLAB:
                        pending = _issue_slab(s + 1)
                    b = s * SLAB + bi
                    px = px_pool.tile([NC2, 2, 8, 64], f32, tag="px")
                    x_sb = work.tile([NC2, 2, 8, 32], bf16, tag="x")
                    xts = [[None, None], [None, None]]
                    # scores + exp per head pair, then attn@V; normalize +
                    # transpose per 4-head half as soon as its pairs finish
                    for g in range(4):
                        ps = ps_pool.tile([NC2, 4, 256], f32, tag="ps")
                        for h2 in range(2):
                            h = 2 * g + h2
                            for j in range(2):
                                nc.tensor.matmul(
                                    ps[:, 2 * h2 + j, 0:196],
                                    lhsT=qk_sb[:, 1, bi, h, ts(j, NC2)],
                                    rhs=qk_sb[:, 0, bi, h],
                                    start=True,
                                    stop=True,
                                )
                        pst = work.tile([NC2, 2, 2, 196], bf16, tag="pst")
                        nc.scalar.activation(
                            pst.rearrange("p a b n -> p (a b) n"),
                            ps[:, :, 0:196],
                            mybir.ActivationFunctionType.Exp,
                        )
                        for h2 in range(2):
                            h = 2 * g + h2
                            for i in range(2):
                                for j in range(2):
                                    nc.tensor.matmul(
                                        px[:, i, h, 0:33],
                                        lhsT=pst[:, h2, j, ts(i, NC2)],
                                        rhs=vx_sb[:, bi, j, h],
                                        start=(j == 0),
                                        stop=(j == 1),
                                    )
                        if g % 2 == 1:
                            # heads 4*half .. 4*half+3 are done: normalize and
                            # transpose that half now to shorten the batch tail
                            half = g // 2
                            hs = ds(4 * half, 4)
                            rc = work.tile([NC2, 2, 4], f32, tag=f"rc{half}")
                            nc.vector.reciprocal_approx_fast(
                                rc[:], px[:, :, hs, 32]
                            )
                            nc.vector.tensor_mul(
                                out=x_sb[:, :, hs],
                                in0=px[:, :, hs, 0:32],
                                in1=rc[:, :, :, None].to_broadcast([NC2, 2, 4, 32]),
                            )
                            for i in range(2):
                                xr = x_sb[:, i].rearrange("p h d -> p (h d)")
                                pt = pt_pool.tile([128, NC2], bf16, tag="pt")
                                nc.tensor.transpose(
                                    pt[:], xr[:, ts(half, 128)], id_sb[:]
                                )
                                xt = work.tile([128, NC2], bf16, tag=f"xt{i}{half}")
                                nc.vector.tensor_copy(xt[:], pt[:])
                                xts[i][half] = xt

                    # proj + bias; outputs for two batches share one DMA
                    if bi % 2 == 0:
                        o2_sb = work.tile([NC2, 2, 2, 256], bf16, tag="o")
                    for i in range(2):
                        po = po_pool.tile([NC2, 256], f32, tag="po")
                        nc.tensor.matmul(
                            po[:], lhsT=ones_sb[:], rhs=pb_sb[:], start=True, stop=False
                        )
                        for half in range(2):
                            nc.tensor.matmul(
                                po[:],
                                lhsT=xts[i][half][:],
                                rhs=w_sb[:, half],
                                start=False,
                                stop=(half == 1),
                            )
                        nc.vector.tensor_copy(o2_sb[:, bi % 2, i], po[:])
                    if bi % 2 == 1:
                        nc.sync.dma_start(
                            out_d[ds(b - 1, 2)].rearrange("b (i p) c -> p b i c", i=2),
                            o2_sb[:],
                        )

    nc.compile()
    _CACHED[key] = nc
    return nc


def _prep_host(q, k, v, dpb_w1, dpb_b1, dpb_w2, dpb_b2, proj_w, proj_b):
    scale = HD ** -0.5
    # rpb via MLP on host, then rank-RPB_R factorization rpb[h].T ~= A_h.T B_h
    biases = _bias_coords(G)
    pos = np.maximum(biases @ dpb_w1 + dpb_b1, 0.0) @ dpb_w2 + dpb_b2  # [729, 8]
    idx = _relative_position_index(G).reshape(-1)
    rpb = pos[idx].reshape(N, N, NH).transpose(2, 0, 1)  # [H, n, m]
    A = np.empty((NH, RPB_R, N), np.float32)  # m side
    Bf = np.empty((NH, RPB_R, N), np.float32)  # n side
    for h in range(NH):
        U, S, Vt = np.linalg.svd(rpb[h], full_matrices=False)
        r = RPB_R
        ss = np.sqrt(S[:r])
        Bf[h] = (U[:, :r] * ss[None, :]).T  # [r, n]
        A[h] = Vt[:r] * ss[:, None]  # [r, m]
    # combined q/k, partition-major [2, KEXT, B, 8, 196]:
    #   rows 0:32  -> q (scaled) / k values per head-dim
    #   rows 32:64 -> B_h (n side) / A_h (m side), replicated across b
    qk = np.empty((2, KEXT, B, 8, 196), np.float32)
    qs = (q.astype(np.float32) * scale).transpose(0, 2, 1).reshape(B, 8, 32, 196)
    qk[0, :32] = qs.transpose(2, 0, 1, 3)
    qk[0, 32:] = Bf.transpose(1, 0, 2)[:, None, :, :]  # [r, 1, h, n] bcast over b
    ks = k.astype(np.float32).transpose(0, 2, 1).reshape(B, 8, 32, 196)
    qk[1, :32] = ks.transpose(2, 0, 1, 3)
    qk[1, 32:] = A.transpose(1, 0, 2)[:, None, :, :]
    qk = qk.astype(BF16)
    # vx partition-major [98, B, 2, 8, 33]
    vr = v.reshape(B, 2, NC2, 8, 32)  # [b, j, p, h, d]
    vx = np.concatenate([vr, np.ones(vr.shape[:-1] + (1,), np.float32)], axis=-1)
    vx = np.ascontiguousarray(vx.transpose(2, 0, 1, 3, 4)).astype(BF16)
    w = np.ascontiguousarray(
        proj_w.reshape(2, 128, 256).transpose(1, 0, 2)
    ).astype(BF16)
    pb = proj_b.reshape(1, 256).astype(BF16)
    ident = np.eye(NC2, dtype=np.float32).astype(BF16)
    return qk, vx, w, pb, ident


def _make_in_maps(inputs) -> list:
    q = np.asarray(inputs["q"], np.float32)
    k = np.asarray(inputs["k"], np.float32)
    v = np.asarray(inputs["v"], np.float32)
    qk, vx, w, pb, ident = _prep_host(
        q, k, v,
        np.asarray(inputs["dpb_w1"], np.float32),
        np.asarray(inputs["dpb_b1"], np.float32),
        np.asarray(inputs["dpb_w2"], np.float32),
        np.asarray(inputs["dpb_b2"], np.float32),
        np.asarray(inputs["proj_w"], np.float32),
        np.asarray(inputs["proj_b"], np.float32),
    )
    in_maps = []
    for c in range(NCORES):
        sl = slice(c * BLOC, (c + 1) * BLOC)
        in_maps.append(
            {
                "qk": np.ascontiguousarray(qk[:, :, sl]),
                "vx": np.ascontiguousarray(vx[:, sl]),
                "w": w,
                "pb": pb,
                "ident": ident,
            }
        )
    return in_maps


def _assemble_out(results) -> np.ndarray:
    out = np.concatenate([r["out"] for r in results], axis=0)
    return out.astype(np.float32)


def kernel(**inputs) -> np.ndarray:
    in_maps = _make_in_maps(inputs)
    nc = _build_bass()
    res = run_bass_kernel_spmd(nc, in_maps, core_ids=list(range(NCORES)))
    _CACHED["last_results"] = res
    return _assemble_out(res.results)


if __name__ == "__main__":
    rng = np.random.default_rng(0)
    ins = {
        "q": rng.standard_normal((B, N, DIM), dtype=np.float32),
        "k": rng.standard_normal((B, N, DIM), dtype=np.float32),
        "v": rng.standard_normal((B, N, DIM), dtype=np.float32),
        "dpb_w1": rng.standard_normal((2, 64), dtype=np.float32) * 0.1,
        "dpb_b1": np.zeros(64, np.float32),
        "dpb_w2": rng.standard_normal((64, 8), dtype=np.float32) * 0.1,
        "dpb_b2": np.zeros(8, np.float32),
        "proj_w": rng.standard_normal((256, 256), dtype=np.float32) * (256 ** -0.5),
        "proj_b": np.zeros(256, np.float32),
        "group_size": 14,
    }
    o = kernel(**ins)
    print(o.shape, o.dtype)



# revision 9
# speedup vs baseline: 1.4909x; 1.4909x over previous
import sys

for p in ("/opt/trn_rl_repo",):
    if p not in sys.path:
        sys.path.insert(0, p)

import numpy as np
import ml_dtypes

import concourse.bass as bass
from concourse import bacc
import concourse.mybir as mybir
import concourse.tile as tile
from concourse.bass import ds, ts
from concourse.bass_utils import run_bass_kernel_spmd

BF16 = ml_dtypes.bfloat16

B, N, DIM, NH = 256, 196, 256, 8
HD = DIM // NH  # 32
G = 14
NCORES = 8
BLOC = B // NCORES  # 32
NC2 = 98  # N / 2
SLAB = 4  # batches per input-DMA slab
NSLAB = BLOC // SLAB
RPB_R = 32  # rank of the rpb factorization folded into the QK matmul
KEXT = HD + RPB_R  # 64


def _relative_position_index(g: int) -> np.ndarray:
    coords = np.stack(np.meshgrid(np.arange(g), np.arange(g), indexing="ij"))
    cf = coords.reshape(2, -1)
    rel = cf[:, :, None] - cf[:, None, :]
    rel = rel.transpose(1, 2, 0).astype(np.int64)
    rel[..., 0] += g - 1
    rel[..., 1] += g - 1
    rel[..., 0] *= 2 * g - 1
    return rel.sum(-1)


def _bias_coords(g: int) -> np.ndarray:
    p = np.arange(1 - g, g)
    biases = np.stack(np.meshgrid(p, p, indexing="ij"))
    return biases.reshape(2, -1).T.astype(np.float32)


_CACHED = {}


def _build_bass(reps: int = 1):
    key = ("nc", reps)
    if key in _CACHED:
        return _CACHED[key]
    f32 = mybir.dt.float32
    bf16 = mybir.dt.bfloat16

    nc = bacc.Bacc("TRN2", target_bir_lowering=False)
    # qk: partition p = kx (0:32 q*scale / k per head-dim, 32:64 the
    # rank-32 rpb factors: n-side in the q half, m-side in the k half).
    qk_d = nc.dram_tensor("qk", [KEXT, BLOC, 2, 8, 196], bf16, kind="ExternalInput")
    v_d = nc.dram_tensor("v", [NC2, BLOC, 2, 8, HD], bf16, kind="ExternalInput")
    w_d = nc.dram_tensor("w", [128, 2, 2, 128], bf16, kind="ExternalInput")
    pb_d = nc.dram_tensor("pb", [1, 2, 128], bf16, kind="ExternalInput")
    out_d = nc.dram_tensor("out", [128, BLOC, 2, 196], bf16, kind="ExternalOutput")

    from contextlib import ExitStack

    with tile.TileContext(nc) as tc, ExitStack() as es:
        const = es.enter_context(tc.tile_pool(name="const", bufs=1))
        io = es.enter_context(tc.tile_pool(name="io", bufs=2))
        oio = es.enter_context(tc.tile_pool(name="oio", bufs=2))
        work = es.enter_context(tc.tile_pool(name="work", bufs=2))
        # PSUM budget (8 banks): ps 2x2 + px 1 + den 2x1 + po 1.
        # Every matmul is its own start/stop group (the scheduler freely
        # reorders ready matmuls, and interleaved accumulation groups in one
        # bank corrupt has_written state); j-accumulation happens in the
        # proj matmuls (for x) and in a DVE add (for the denominators).
        ps_pool = es.enter_context(tc.tile_pool(name="ps", bufs=2, space="PSUM"))
        px_pool = es.enter_context(tc.tile_pool(name="px", bufs=1, space="PSUM"))
        den_pool = es.enter_context(tc.tile_pool(name="den", bufs=2, space="PSUM"))
        po_pool = es.enter_context(tc.tile_pool(name="po", bufs=1, space="PSUM"))

        w_sb = const.tile([128, 2, 2, 128], bf16)
        pb_sb = const.tile([1, 2, 128], bf16)
        ones_sb = const.tile([1, 196], bf16)
        ones32_sb = const.tile([NC2, 32], bf16)
        nc.vector.memset(ones_sb[:], 1.0)
        nc.vector.memset(ones32_sb[:], 1.0)

        def _issue_const_dmas():
            nc.sync.dma_start(w_sb[:], w_d[:])
            nc.sync.dma_start(pb_sb[:], pb_d[:])

        for rep in range(reps):
            def _issue_slab(s, first=False):
                qk_sb = io.tile([KEXT, SLAB, 2, 8, 196], bf16, tag="qk")
                v_sb = io.tile([NC2, SLAB, 2, 8, HD], bf16, tag="v")
                nc.sync.dma_start(qk_sb[:], qk_d[:, ds(s * SLAB, SLAB)])
                nc.sync.dma_start(v_sb[:], v_d[:, ds(s * SLAB, SLAB)])
                if first:
                    _issue_const_dmas()
                return qk_sb, v_sb

            pending = _issue_slab(0, first=(rep == 0))
            for s in range(NSLAB):
                qk_sb, v_sb = pending
                out_sb = oio.tile([128, SLAB, 2, 196], bf16, tag="o")
                for bi in range(SLAB):
                    if bi == 1 and s + 1 < NSLAB:
                        pending = _issue_slab(s + 1)
                    # scores (transposed: [m, n]) + exp, in 4 chunks of
                    # (j m-half, g head-quad); heads pair-packed in the PE
                    # array rows via tile_position (K=64 each).
                    pst = work.tile([NC2, 2, 8, 196], bf16, tag="pst")
                    for j in (0, 1):
                        for g in (0, 1):
                            ps = ps_pool.tile(
                                [NC2, 2, 2, 196], f32, tag="ps",
                                padded_shape=[None, None, None, 256],
                            )
                            for hl in (0, 1):
                                for e in (0, 1):
                                    h = 4 * g + 2 * hl + e
                                    nc.tensor.matmul(
                                        ps[:, hl, e, :],
                                        lhsT=qk_sb[:, bi, 1, h, ds(98 * j, NC2)],
                                        rhs=qk_sb[:, bi, 0, h, :],
                                        start=True,
                                        stop=True,
                                    )
                            nc.scalar.activation(
                                pst[:, j, ds(4 * g, 4), :],
                                ps[:].rearrange("p a e n -> p (a e) n"),
                                mybir.ActivationFunctionType.Exp,
                            )
                    # softmax denominators: ones-matmuls replicate each
                    # head's denominator across its 32 proj rows; j-chunks
                    # land in separate banks, summed on the DVE.
                    den_j0 = den_pool.tile([128, 2, 196], f32, tag="den", padded_shape=[None, None, 256])
                    den_j1 = den_pool.tile([128, 2, 196], f32, tag="den", padded_shape=[None, None, 256])
                    for j, den in ((0, den_j0), (1, den_j1)):
                        for g in (0, 1):
                            for hl in range(4):
                                nc.tensor.matmul(
                                    den[ds(32 * hl, 32), g, :],
                                    lhsT=ones32_sb[:],
                                    rhs=pst[:, j, 4 * g + hl, :],
                                    start=True,
                                    stop=True,
                                    tile_position=(0, 32 * hl),
                                )
                    d0s = work.tile([128, 2, 196], f32, tag="d0s")
                    nc.vector.tensor_copy(d0s[:], den_j0[:])
                    dsum = work.tile([128, 2, 196], f32, tag="dsum")
                    nc.vector.tensor_add(dsum[:], den_j1[:], d0s[:])
                    rc = work.tile([128, 2, 196], f32, tag="rc")
                    nc.vector.reciprocal_approx_fast(rc[:], dsum[:])
                    # attn@V with V stationary: x^T lands as [c, n] = proj
                    # lhsT layout (no transpose needed); 4 heads col-packed.
                    # One px bank, reused j0 -> j1 (WAR via the mul).
                    x_sb = work.tile([128, 2, 2, 196], bf16, tag="x")
                    px = px_pool.tile([128, 2, 196], f32, tag="px", padded_shape=[None, None, 256])
                    for j in (0, 1):
                        for g in (0, 1):
                            for hl in range(4):
                                nc.tensor.matmul(
                                    px[ds(32 * hl, 32), g, :],
                                    lhsT=v_sb[:, bi, j, 4 * g + hl, :],
                                    rhs=pst[:, j, 4 * g + hl, :],
                                    start=True,
                                    stop=True,
                                    tile_position=(0, 32 * hl),
                                )
                        nc.vector.tensor_mul(out=x_sb[:, j], in0=px[:], in1=rc[:])
                    # proj in out^T orientation: po[co, n] = sum over (kc, j)
                    # of W-chunk^T @ x^T_j, plus bias; one po bank reused
                    # across oc (WAR via the copy).
                    for oc in (0, 1):
                        po = po_pool.tile([128, 196], f32, tag="po", padded_shape=[None, 512])
                        nc.tensor.matmul(
                            po[:],
                            lhsT=pb_sb[:, oc, :],
                            rhs=ones_sb[:],
                            start=True,
                            stop=False,
                        )
                        for kc in (0, 1):
                            for j in (0, 1):
                                nc.tensor.matmul(
                                    po[:],
                                    lhsT=w_sb[:, kc, oc, :],
                                    rhs=x_sb[:, j, kc, :],
                                    start=False,
                                    stop=(kc == 1 and j == 1),
                                )
                        nc.vector.tensor_copy(out_sb[:, bi, oc, :], po[:])
                nc.sync.dma_start(out_d[:, ds(s * SLAB, SLAB)], out_sb[:])

    nc.compile()
    _CACHED[key] = nc
    return nc


def _prep_host(q, k, v, dpb_w1, dpb_b1, dpb_w2, dpb_b2, proj_w, proj_b):
    scale = HD ** -0.5
    # rpb via MLP on host, then rank-RPB_R factorization rpb[h] ~= Bf_h.T A_h
    biases = _bias_coords(G)
    pos = np.maximum(biases @ dpb_w1 + dpb_b1, 0.0) @ dpb_w2 + dpb_b2  # [729, 8]
    idx = _relative_position_index(G).reshape(-1)
    rpb = pos[idx].reshape(N, N, NH).transpose(2, 0, 1)  # [H, n, m]
    A = np.empty((NH, RPB_R, N), np.float32)  # m side
    Bf = np.empty((NH, RPB_R, N), np.float32)  # n side
    for h in range(NH):
        U, S, Vt = np.linalg.svd(rpb[h], full_matrices=False)
        r = RPB_R
        ss = np.sqrt(S[:r])
        Bf[h] = (U[:, :r] * ss[None, :]).T  # [r, n]
        A[h] = Vt[:r] * ss[:, None]  # [r, m]
    # qk [128, B, 2, 4, 196]: partition p = 64*e + kx, head h = 2*hp + e
    qs = (q.astype(np.float32) * scale).transpose(0, 2, 1).reshape(B, 8, HD, N)
    ks = k.astype(np.float32).transpose(0, 2, 1).reshape(B, 8, HD, N)
    qk = np.empty((KEXT, B, 2, 8, N), np.float32)
    qk[:HD, :, 0] = qs.transpose(2, 0, 1, 3)
    qk[HD:, :, 0] = Bf.transpose(1, 0, 2)[:, None]
    qk[:HD, :, 1] = ks.transpose(2, 0, 1, 3)
    qk[HD:, :, 1] = A.transpose(1, 0, 2)[:, None]
    qk = qk.astype(BF16)
    # v [98, B, 2, 8, 32]
    vr = (
        v.astype(np.float32)
        .reshape(B, 2, NC2, 8, HD)
        .transpose(2, 0, 1, 3, 4)
    )
    vx = np.ascontiguousarray(vr).astype(BF16)
    # w [128, 2, 2, 128]: w[p, kc, oc, co] = proj_w[128*kc + p, 128*oc + co]
    w = np.ascontiguousarray(
        proj_w.astype(np.float32).reshape(2, 128, 2, 128).transpose(1, 0, 2, 3)
    ).astype(BF16)
    pb = proj_b.astype(np.float32).reshape(1, 2, 128).astype(BF16)
    return qk, vx, w, pb


def _make_in_maps(inputs) -> list:
    q = np.asarray(inputs["q"], np.float32)
    k = np.asarray(inputs["k"], np.float32)
    v = np.asarray(inputs["v"], np.float32)
    qk, vx, w, pb = _prep_host(
        q, k, v,
        np.asarray(inputs["dpb_w1"], np.float32),
        np.asarray(inputs["dpb_b1"], np.float32),
        np.asarray(inputs["dpb_w2"], np.float32),
        np.asarray(inputs["dpb_b2"], np.float32),
        np.asarray(inputs["proj_w"], np.float32),
        np.asarray(inputs["proj_b"], np.float32),
    )
    in_maps = []
    for c in range(NCORES):
        sl = slice(c * BLOC, (c + 1) * BLOC)
        in_maps.append(
            {
                "qk": np.ascontiguousarray(qk[:, sl]),
                "v": np.ascontiguousarray(vx[:, sl]),
                "w": w,
                "pb": pb,
            }
        )
    return in_maps


def _assemble_out(results) -> np.ndarray:
    # per-core out [128, BLOC, 2, 196] -> [BLOC, 196, 256]
    outs = []
    for r in results:
        o = np.asarray(r["out"]).astype(np.float32)  # [128, BLOC, 2, 196]
        outs.append(o.transpose(1, 3, 2, 0).reshape(BLOC, N, DIM))
    return np.concatenate(outs, axis=0)


def kernel(**inputs) -> np.ndarray:
    in_maps = _make_in_maps(inputs)
    nc = _build_bass()
    res = run_bass_kernel_spmd(nc, in_maps, core_ids=list(range(NCORES)))
    _CACHED["last_results"] = res
    return _assemble_out(res.results)


if __name__ == "__main__":
    rng = np.random.default_rng(0)
    ins = {
        "q": rng.standard_normal((B, N, DIM), dtype=np.float32),
        "k": rng.standard_normal((B, N, DIM), dtype=np.float32),
        "v": rng.standard_normal((B, N, DIM), dtype=np.float32),
        "dpb_w1": rng.standard_normal((2, 64), dtype=np.float32) * 0.1,
        "dpb_b1": np.zeros(64, np.float32),
        "dpb_w2": rng.standard_normal((64, 8), dtype=np.float32) * 0.1,
        "dpb_b2": np.zeros(8, np.float32),
        "proj_w": rng.standard_normal((256, 256), dtype=np.float32) * (256 ** -0.5),
        "proj_b": np.zeros(256, np.float32),
        "group_size": 14,
    }
    o = kernel(**ins)
    print(o.shape, o.dtype)


# revision 13
# speedup vs baseline: 1.5930x; 1.0685x over previous
import sys

for p in ("/opt/trn_rl_repo",):
    if p not in sys.path:
        sys.path.insert(0, p)

import numpy as np
import ml_dtypes

import concourse.bass as bass
from concourse import bacc
import concourse.mybir as mybir
import concourse.tile as tile
from concourse.bass import ds, ts
from concourse.bass_utils import run_bass_kernel_spmd

BF16 = ml_dtypes.bfloat16

B, N, DIM, NH = 256, 196, 256, 8
HD = DIM // NH  # 32
G = 14
NCORES = 8
BLOC = B // NCORES  # 32
NC2 = 98  # N / 2
SLAB = 4  # batches per input-DMA slab
NSLAB = BLOC // SLAB
RPB_R = 32  # rank of the rpb factorization folded into the QK matmul
KEXT = HD + RPB_R  # 64


def _relative_position_index(g: int) -> np.ndarray:
    coords = np.stack(np.meshgrid(np.arange(g), np.arange(g), indexing="ij"))
    cf = coords.reshape(2, -1)
    rel = cf[:, :, None] - cf[:, None, :]
    rel = rel.transpose(1, 2, 0).astype(np.int64)
    rel[..., 0] += g - 1
    rel[..., 1] += g - 1
    rel[..., 0] *= 2 * g - 1
    return rel.sum(-1)


def _bias_coords(g: int) -> np.ndarray:
    p = np.arange(1 - g, g)
    biases = np.stack(np.meshgrid(p, p, indexing="ij"))
    return biases.reshape(2, -1).T.astype(np.float32)


_CACHED = {}


def _build_bass(reps: int = 1):
    key = ("nc", reps)
    if key in _CACHED:
        return _CACHED[key]
    f32 = mybir.dt.float32
    bf16 = mybir.dt.bfloat16

    nc = bacc.Bacc("TRN2", target_bir_lowering=False)
    # qk: partition p = kx (0:32 q*scale / k per head-dim, 32:64 the
    # rank-32 rpb factors: n-side in the q half, m-side in the k half).
    qk_d = nc.dram_tensor("qk", [KEXT, BLOC, 2, 8, 196], bf16, kind="ExternalInput")
    v_d = nc.dram_tensor("v", [NC2, BLOC, 2, 8, HD], bf16, kind="ExternalInput")
    w_d = nc.dram_tensor("w", [128, 2, 2, 128], bf16, kind="ExternalInput")
    pb_d = nc.dram_tensor("pb", [1, 2, 128], bf16, kind="ExternalInput")
    out_d = nc.dram_tensor("out", [128, BLOC, 2, 196], bf16, kind="ExternalOutput")

    from contextlib import ExitStack

    with tile.TileContext(nc) as tc, ExitStack() as es:
        const = es.enter_context(tc.tile_pool(name="const", bufs=1))
        io = es.enter_context(tc.tile_pool(name="io", bufs=2))
        oio = es.enter_context(tc.tile_pool(name="oio", bufs=2))
        work = es.enter_context(tc.tile_pool(name="work", bufs=2))
        # PSUM budget (8 banks): ps 2x2 + px 1 + den 1 + po 2x1.
        # Every matmul is its own start/stop group (the scheduler freely
        # reorders ready matmuls, and interleaved accumulation groups in one
        # bank corrupt has_written state); j-accumulation happens in the
        # proj matmuls (for x) and in a DVE add (for the denominators).
        ps_pool = es.enter_context(tc.tile_pool(name="ps", bufs=2, space="PSUM"))
        px_pool = es.enter_context(tc.tile_pool(name="px", bufs=1, space="PSUM"))
        den_pool = es.enter_context(tc.tile_pool(name="den", bufs=1, space="PSUM"))
        po_pool = es.enter_context(tc.tile_pool(name="po", bufs=2, space="PSUM"))

        w_sb = const.tile([128, 2, 2, 128], bf16)
        pb_sb = const.tile([1, 2, 128], bf16)
        ones_sb = const.tile([1, 196], bf16)
        ones32_sb = const.tile([NC2, 32], bf16)
        nc.vector.memset(ones_sb[:], 1.0)
        nc.vector.memset(ones32_sb[:], 1.0)

        def _issue_const_dmas():
            nc.sync.dma_start(w_sb[:], w_d[:])
            nc.sync.dma_start(pb_sb[:], pb_d[:])

        for rep in range(reps):
            def _issue_slab(s, first=False):
                qk_sb = io.tile([KEXT, SLAB, 2, 8, 196], bf16, tag="qk")
                v_sb = io.tile([NC2, SLAB, 2, 8, HD], bf16, tag="v")
                nc.sync.dma_start(qk_sb[:], qk_d[:, ds(s * SLAB, SLAB)])
                nc.sync.dma_start(v_sb[:], v_d[:, ds(s * SLAB, SLAB)])
                if first:
                    _issue_const_dmas()
                return qk_sb, v_sb

            pending = _issue_slab(0, first=(rep == 0))
            for s in range(NSLAB):
                qk_sb, v_sb = pending
                out_sb = oio.tile([128, SLAB, 2, 196], bf16, tag="o")
                for bi in range(SLAB):
                    if bi == 1 and s + 1 < NSLAB:
                        pending = _issue_slab(s + 1)
                    # scores (transposed: [m, n]) + exp, in 4 chunks of
                    # (j m-half, g head-quad); per-j pst tiles so AV/den of
                    # j0 can overlap the j1 exp chunks.
                    pst = [
                        work.tile([NC2, 8, 196], bf16, tag="pst0", name="pst0"),
                        work.tile([NC2, 8, 196], bf16, tag="pst1", name="pst1"),
                    ]
                    den = [
                        den_pool.tile([128, 2, 196], f32, tag="den", name="den0",
                                      padded_shape=[None, None, 256]),
                        den_pool.tile([128, 2, 196], f32, tag="den", name="den1",
                                      padded_shape=[None, None, 256]),
                    ]
                    d0s = work.tile([128, 2, 196], f32, tag="d0s")
                    dsum = work.tile([128, 2, 196], f32, tag="dsum")
                    rc = work.tile([128, 2, 196], f32, tag="rc")
                    for j in (0, 1):
                        for g in (0, 1):
                            ps = ps_pool.tile(
                                [NC2, 2, 2, 196], f32, tag="ps",
                                padded_shape=[None, None, None, 256],
                            )
                            for hl in (0, 1):
                                for e in (0, 1):
                                    h = 4 * g + 2 * hl + e
                                    nc.tensor.matmul(
                                        ps[:, hl, e, :],
                                        lhsT=qk_sb[:, bi, 1, h, ds(98 * j, NC2)],
                                        rhs=qk_sb[:, bi, 0, h, :],
                                        start=True,
                                        stop=True,
                                    )
                            nc.scalar.activation(
                                pst[j][:, ds(4 * g, 4), :],
                                ps[:].rearrange("p a e n -> p (a e) n"),
                                mybir.ActivationFunctionType.Exp,
                            )
                        # softmax denominators: ones-matmuls replicate each
                        # head's denominator across its 32 proj rows.
                        for g in (0, 1):
                            for hl in range(4):
                                nc.tensor.matmul(
                                    den[j][ds(32 * hl, 32), g, :],
                                    lhsT=ones32_sb[:],
                                    rhs=pst[j][:, 4 * g + hl, :],
                                    start=True,
                                    stop=True,
                                    tile_position=(0, 32 * hl),
                                )
                        if j == 0:
                            nc.vector.tensor_copy(d0s[:], den[0][:])
                    nc.vector.tensor_add(dsum[:], den[1][:], d0s[:])
                    nc.vector.reciprocal_approx_fast(rc[:], dsum[:])
                    # attn@V with V stationary: x^T lands as [c, n] = proj
                    # lhsT layout (no transpose needed); 4 heads col-packed.
                    # One px bank, reused j0 -> j1 (WAR via the mul).
                    x_sb = work.tile([128, 2, 2, 196], bf16, tag="x")
                    px = px_pool.tile([128, 2, 196], f32, tag="px", padded_shape=[None, None, 256])
                    for j in (0, 1):
                        for g in (0, 1):
                            for hl in range(4):
                                nc.tensor.matmul(
                                    px[ds(32 * hl, 32), g, :],
                                    lhsT=v_sb[:, bi, j, 4 * g + hl, :],
                                    rhs=pst[j][:, 4 * g + hl, :],
                                    start=True,
                                    stop=True,
                                    tile_position=(0, 32 * hl),
                                )
                        nc.vector.tensor_mul(out=x_sb[:, j], in0=px[:], in1=rc[:])
                    xs = work.tile([128, 2, 196], bf16, tag="xs")
                    nc.vector.tensor_add(xs[:], x_sb[:, 0], x_sb[:, 1])
                    # proj in out^T orientation: po[co, n] = sum over kc of
                    # W-chunk^T @ (x^T_j0 + x^T_j1), plus bias.
                    for oc in (0, 1):
                        po = po_pool.tile([128, 196], f32, tag="po", padded_shape=[None, 512])
                        nc.tensor.matmul(
                            po[:],
                            lhsT=pb_sb[:, oc, :],
                            rhs=ones_sb[:],
                            start=True,
                            stop=False,
                        )
                        for kc in (0, 1):
                            nc.tensor.matmul(
                                po[:],
                                lhsT=w_sb[:, kc, oc, :],
                                rhs=xs[:, kc, :],
                                start=False,
                                stop=(kc == 1),
                            )
                        nc.vector.tensor_copy(out_sb[:, bi, oc, :], po[:])
                nc.sync.dma_start(out_d[:, ds(s * SLAB, SLAB)], out_sb[:])

    nc.compile()
    _CACHED[key] = nc
    return nc


def _prep_host(q, k, v, dpb_w1, dpb_b1, dpb_w2, dpb_b2, proj_w, proj_b):
    scale = HD ** -0.5
    # rpb via MLP on host, then rank-RPB_R factorization rpb[h] ~= Bf_h.T A_h
    biases = _bias_coords(G)
    pos = np.maximum(biases @ dpb_w1 + dpb_b1, 0.0) @ dpb_w2 + dpb_b2  # [729, 8]
    idx = _relative_position_index(G).reshape(-1)
    rpb = pos[idx].reshape(N, N, NH).transpose(2, 0, 1)  # [H, n, m]
    A = np.empty((NH, RPB_R, N), np.float32)  # m side
    Bf = np.empty((NH, RPB_R, N), np.float32)  # n side
    for h in range(NH):
        U, S, Vt = np.linalg.svd(rpb[h], full_matrices=False)
        r = RPB_R
        ss = np.sqrt(S[:r])
        Bf[h] = (U[:, :r] * ss[None, :]).T  # [r, n]
        A[h] = Vt[:r] * ss[:, None]  # [r, m]
    # qk [128, B, 2, 4, 196]: partition p = 64*e + kx, head h = 2*hp + e
    qs = (q.astype(np.float32) * scale).transpose(0, 2, 1).reshape(B, 8, HD, N)
    ks = k.astype(np.float32).transpose(0, 2, 1).reshape(B, 8, HD, N)
    qk = np.empty((KEXT, B, 2, 8, N), np.float32)
    qk[:HD, :, 0] = qs.transpose(2, 0, 1, 3)
    qk[HD:, :, 0] = Bf.transpose(1, 0, 2)[:, None]
    qk[:HD, :, 1] = ks.transpose(2, 0, 1, 3)
    qk[HD:, :, 1] = A.transpose(1, 0, 2)[:, None]
    qk = qk.astype(BF16)
    # v [98, B, 2, 8, 32]
    vr = (
        v.astype(np.float32)
        .reshape(B, 2, NC2, 8, HD)
        .transpose(2, 0, 1, 3, 4)
    )
    vx = np.ascontiguousarray(vr).astype(BF16)
    # w [128, 2, 2, 128]: w[p, kc, oc, co] = proj_w[128*kc + p, 128*oc + co]
    w = np.ascontiguousarray(
        proj_w.astype(np.float32).reshape(2, 128, 2, 128).transpose(1, 0, 2, 3)
    ).astype(BF16)
    pb = proj_b.astype(np.float32).reshape(1, 2, 128).astype(BF16)
    return qk, vx, w, pb


def _make_in_maps(inputs) -> list:
    q = np.asarray(inputs["q"], np.float32)
    k = np.asarray(inputs["k"], np.float32)
    v = np.asarray(inputs["v"], np.float32)
    qk, vx, w, pb = _prep_host(
        q, k, v,
        np.asarray(inputs["dpb_w1"], np.float32),
        np.asarray(inputs["dpb_b1"], np.float32),
        np.asarray(inputs["dpb_w2"], np.float32),
        np.asarray(inputs["dpb_b2"], np.float32),
        np.asarray(inputs["proj_w"], np.float32),
        np.asarray(inputs["proj_b"], np.float32),
    )
    in_maps = []
    for c in range(NCORES):
        sl = slice(c * BLOC, (c + 1) * BLOC)
        in_maps.append(
            {
                "qk": np.ascontiguousarray(qk[:, sl]),
                "v": np.ascontiguousarray(vx[:, sl]),
                "w": w,
                "pb": pb,
            }
        )
    return in_maps


def _assemble_out(results) -> np.ndarray:
    # per-core out [128, BLOC, 2, 196] -> [BLOC, 196, 256]
    outs = []
    for r in results:
        o = np.asarray(r["out"]).astype(np.float32)  # [128, BLOC, 2, 196]
        outs.append(o.transpose(1, 3, 2, 0).reshape(BLOC, N, DIM))
    return np.concatenate(outs, axis=0)


def kernel(**inputs) -> np.ndarray:
    in_maps = _make_in_maps(inputs)
    nc = _build_bass()
    res = run_bass_kernel_spmd(nc, in_maps, core_ids=list(range(NCORES)))
    _CACHED["last_results"] = res
    return _assemble_out(res.results)


if __name__ == "__main__":
    rng = np.random.default_rng(0)
    ins = {
        "q": rng.standard_normal((B, N, DIM), dtype=np.float32),
        "k": rng.standard_normal((B, N, DIM), dtype=np.float32),
        "v": rng.standard_normal((B, N, DIM), dtype=np.float32),
        "dpb_w1": rng.standard_normal((2, 64), dtype=np.float32) * 0.1,
        "dpb_b1": np.zeros(64, np.float32),
        "dpb_w2": rng.standard_normal((64, 8), dtype=np.float32) * 0.1,
        "dpb_b2": np.zeros(8, np.float32),
        "proj_w": rng.standard_normal((256, 256), dtype=np.float32) * (256 ** -0.5),
        "proj_b": np.zeros(256, np.float32),
        "group_size": 14,
    }
    o = kernel(**ins)
    print(o.shape, o.dtype)
